# revision 68
# baseline (speedup 1.0000x reference)
"""Trainium2 Bass kernel for multi-head causal attention.

Problem: B=2, S=2048, D=1024, H=16 heads (head_dim=64), fp32.
  q,k,v = x@Wq, x@Wk, x@Wv  (per-head split)
  scores = q@k^T, causal mask, softmax(scores/sqrt(64))
  out = (attn@v concat) @ Wo + bo

Sharding (8 cores): core c -> batch b=c//4, head group g=c%4 (4 heads).
Each core computes its 4 heads' attention plus the partial output
projection (row-parallel Wo); host sums 4 partials per batch and adds bo.

Layout strategy (zero on-device transposes):
 - x^T passed host-transposed (feature-major).
 - Q^T,K^T produced feature-major: (head_dim x tokens), two heads stacked
   per 128-partition tile; scores^T computed per 64-partition row group
   (the PE runs the two 64-row matmuls concurrently in row groups).
 - Both heads' scores^T tiles (k x q) land in one 2-bank PSUM tile so the
   exp runs as a single wide instruction. The exp'd bf16 tile is directly
   the PV stationary operand. V is token-major with an appended ones
   column so the PV matmul also emits the softmax denominators.
 - causal structure: diagonal 128x512 blocks trim their scores / exp /
   PV widths to the valid q range; the 128x128 triangle on the diagonal
   is zeroed post-exp with one gpsimd affine_select (no mask adds, no
   memsets, nothing on the DVE or scalar engines).
 - scalar engine runs ONLY the exp chain mid-kernel; everything else
   (projection staging, accumulator staging, normalization, Wo staging)
   is on DVE/gpsimd/DMA so softmax never waits.
 - emission is a global software pipeline: scores for block i+1 and
   filler matmuls (QKV projections for range r+1, Wo for range r-1) are
   emitted between scores(i) and PV(i), so the in-order PE queue always
   has ~1us of exp-independent work in front of each PV and never stalls
   on the softmax chain. K/V chains for range r+1 are held back and
   drained inside range r+1 (before its diagonal blocks need them) so the
   late, exp-starved ranges still have PE filler. PSUM: 2x2 banks
   scores, 2 banks PV accum, 2 banks projections = 8; the dead scores
   banks are recycled for the tail Wo chains.
 - normalization: denominator row hops PSUM partition 64 -> SBUF
   partition 0 with one DVE copy (32-aligned cross-partition-base
   operands are legal), then fast reciprocal -> gpsimd partition
   broadcast -> multiply, deferred one step per block so the cross-engine
   latency never blocks the in-order queues. The gpsimd broadcast
   library is warm-loaded at t=0 (the ~7us load hides under the initial
   DMA wait).
 - startup: weight/x DMAs are split into many descriptors, spread over
   the sync/scalar/gpsimd queues in need-order (each descriptor costs
   ~600ns of issuing-engine time), and x range 1 is sequenced behind
   range 0 so it doesn't steal DMA bandwidth from the critical path.
Matmul operands are bf16 (1 cycle/row PE rate); every accumulation and
the softmax normalization stay fp32 in PSUM.
"""

import sys

sys.path.insert(0, "/opt/trn_rl_repo")

from collections import deque

import ml_dtypes
import numpy as np

import concourse.bass as bass  # noqa: F401
import concourse.tile as tile
from concourse import bacc, bass_utils, mybir

F32 = mybir.dt.float32
MMDT = mybir.dt.bfloat16
NPDT = ml_dtypes.bfloat16
EXPF = mybir.ActivationFunctionType.Exp

B, S, D, H, HD = 2, 2048, 1024, 16, 64
N_CORES = 8
HPC = 4            # heads per core
GW = HPC * HD      # head-group width per core = 256
SCALE = 1.0 / np.sqrt(HD)

_CACHE = {}
LAST_RESULTS = None


def _maybe_install_trace_hook():
    """If BASS_TRACE is set, bass_utils needs antenv.axon_hooks (absent in
    this image). Install it from trn_boot when possible; otherwise disable
    tracing so the run still works."""
    import os

    if not os.environ.get("BASS_TRACE"):
        return
    try:
        import antenv.axon_hooks  # noqa: F401
        return
    except ImportError:
        pass
    try:
        import types

        from trn_agent_boot.trn_boot import _ntff_profile_via_ctypes

        hook = _ntff_profile_via_ctypes("/opt/axon/libaxon_pjrt.so")
        mod = types.ModuleType("antenv.axon_hooks")
        mod.get_axon_ntff_profile_hook = lambda: hook
        mod.set_axon_ntff_profile_hook = lambda h: None
        import antenv

        sys.modules["antenv.axon_hooks"] = mod
        antenv.axon_hooks = mod
    except Exception:
        os.environ["BASS_NEVER_TRACE"] = "1"


def _build():
    nc = bacc.Bacc("TRN2", target_bir_lowering=False, debug=False)

    xT = nc.dram_tensor("xT", [D, S], MMDT, kind="ExternalInput").ap()
    wq = nc.dram_tensor("wq", [128, D // 128 * GW], MMDT, kind="ExternalInput").ap()
    wk = nc.dram_tensor("wk", [128, D // 128 * GW], MMDT, kind="ExternalInput").ap()
    wv = nc.dram_tensor("wv", [128, D // 128 * GW], MMDT, kind="ExternalInput").ap()
    wo = nc.dram_tensor("wo", [128, GW // 128 * D], MMDT, kind="ExternalInput").ap()
    out = nc.dram_tensor("out", [S, D], F32, kind="ExternalOutput").ap()

    NT = S // 512          # 4 q/t ranges of 512
    NC = D // 128          # 8 contraction chunks for projections
    NJ = S // 128          # 16 k-chunks

    with tile.TileContext(nc) as tc, nc.allow_low_precision(reason="bf16 matmuls"):
        with (
            tc.tile_pool(name="const", bufs=1) as cpool,
            tc.tile_pool(name="xin", bufs=3) as xpool,
            tc.tile_pool(name="pt", bufs=8) as ppool,
            tc.tile_pool(name="small", bufs=8) as spool,
            tc.tile_pool(name="ost", bufs=6) as opool,
            tc.tile_pool(name="ps_sc", bufs=1, space="PSUM") as ps_sc,
            tc.tile_pool(name="ps_acc", bufs=1, space="PSUM") as ps_acc,
            tc.tile_pool(name="ps_mm", bufs=1, space="PSUM") as ps_mm,
        ):
            # ---- persistent tiles ----
            wq_sb = cpool.tile([128, NC, GW], MMDT)
            wk_sb = cpool.tile([128, NC, GW], MMDT)
            wv_sb = cpool.tile([128, NC, GW], MMDT)
            wo_sb = cpool.tile([128, 2, D], MMDT)

            QT = cpool.tile([128, 2, S], MMDT)   # [:, pair, t] feature-major
            KT = cpool.tile([128, 2, S], MMDT)
            Vt = cpool.tile([128, NJ, HPC * 65], MMDT)  # token-major + ones col
            ctxT = cpool.tile([128, 2, S], MMDT)

            # ones columns of V (col 64 of each 65-wide head slot)
            vt_ones = Vt[:, :, :].rearrange("p j (h u) -> p (j h) u", u=65)[:, :, 64:65]
            nc.vector.memset(vt_ones, 1.0)
            warm = cpool.tile([1, 8], F32)
            nc.vector.memset(warm[:], 1.0)
            warmb = cpool.tile([2, 8], F32)

            xts = {}

            def load_xt(r, eng=None):
                xt = xpool.tile([128, NC, 512], MMDT, tag="xt")
                xv = xT[:, 512 * r : 512 * (r + 1)].rearrange("(c p) t -> p c t", p=128)
                for c in range(NC):
                    (eng or nc.sync).dma_start(xt[:, c, :], xv[:, c, :])
                xts[r] = xt

            # ---- projection / output chains as (cost, emit) generators ----

            def qk_chain(r, w_sb, dst, o):
                pm = ps_mm.tile([128, 512], F32, tag="mm", bufs=2)
                for c in range(NC):
                    yield 215, lambda c=c: nc.tensor.matmul(
                        pm[:],
                        w_sb[:, c, 128 * o : 128 * (o + 1)],
                        xts[r][:, c, :],
                        start=(c == 0),
                        stop=(c == NC - 1),
                    )
                yield 0, lambda: nc.vector.tensor_copy(
                    dst[:, o, 512 * r : 512 * (r + 1)], pm[:]
                )

            def v_chain(r, tt):
                j = 4 * r + tt
                pv = ps_mm.tile([128, 512], F32, tag="mm", bufs=2)
                for c in range(NC):
                    yield 110, lambda c=c: nc.tensor.matmul(
                        pv[:, 0:GW],
                        xts[r][:, c, 128 * tt : 128 * (tt + 1)],
                        wv_sb[:, c, :],
                        start=(c == 0),
                        stop=(c == NC - 1),
                    )
                yield 0, lambda: nc.vector.tensor_copy(
                    Vt[:, j, :].rearrange("p (h u) -> p h u", u=65)[:, :, 0:64],
                    pv[:, 0:GW].rearrange("p (h d) -> p h d", d=HD),
                )

            def wo_chain(r, qq, o, alt_pool=False, tail_idx=None):
                qt = 4 * r + qq
                if alt_pool:
                    # the scores banks are dead in the tail; reuse them so
                    # four chains pipeline instead of two
                    pot = ps_sc.tile([128, 1024], F32, tag="s2", bufs=2, name="po")
                    po = pot[:, 0:512]
                else:
                    pot = ps_mm.tile([128, 512], F32, tag="mm", bufs=2, name="po")
                    po = pot[:, :]
                for d in range(2):
                    yield 215, lambda d=d: nc.tensor.matmul(
                        po,
                        ctxT[:, d, 128 * qt : 128 * (qt + 1)],
                        wo_sb[:, d, 512 * o : 512 * (o + 1)],
                        start=(d == 0), stop=(d == 1),
                    )

                def fin():
                    ot = opool.tile([128, 512], F32, tag="ot")
                    # in the tail the scalar engine is done with exps and the
                    # DMA dispatch queues are idle: spread staging copies and
                    # output descriptors across engines so chains pipeline
                    if tail_idx is not None and tail_idx % 2 == 0:
                        nc.scalar.copy(ot[:], po)
                    else:
                        nc.vector.tensor_copy(ot[:], po)
                    eng = (
                        nc.sync if tail_idx is None
                        else (nc.scalar, nc.gpsimd, nc.sync)[tail_idx % 3]
                    )
                    eng.dma_start(
                        out[128 * qt : 128 * (qt + 1), 512 * o : 512 * (o + 1)],
                        ot[:],
                    )
                yield 0, fin

            def q_chain_gens(r):
                return [qk_chain(r, wq_sb, QT, o) for o in range(2)]

            def k_chain_gens(r):
                return [qk_chain(r, wk_sb, KT, o) for o in range(2)]

            def v_chain_gens(r):
                return [v_chain(r, tt) for tt in range(4)]

            def c_chain_gens(r):
                return [wo_chain(r, qq, o) for qq in range(4) for o in range(2)]

            # three priorities; generators always run front-to-completion so
            # psum "mm" buffer rotation matches emission order.
            kv_q = deque()      # this range's K/V chains (most urgent)
            urgent_q = deque()  # next range's Q chains (drained by boundary)
            defer_q = deque()   # Wo chains for finished ranges (lazy)

            defer_floor = [0]
            v_q = deque()
            vdone = [0]

            def pop_filler(budget):
                while budget > 0:
                    if v_q:
                        try:
                            cost, emit = next(v_q[0])
                        except StopIteration:
                            v_q.popleft()
                            vdone[0] += 1
                            continue
                        emit()
                        budget -= cost
                        continue
                    q = kv_q or urgent_q
                    if not q:
                        # wo chains read ctxT: make sure every pending
                        # normalization step is emitted first
                        while norm_q:
                            norm_q.popleft()()
                        if len(defer_q) <= defer_floor[0]:
                            return
                        q = defer_q
                    if not q:
                        return
                    try:
                        cost, emit = next(q[0])
                    except StopIteration:
                        q.popleft()
                        continue
                    emit()
                    budget -= cost

            def drain(q):
                while q:
                    try:
                        cost, emit = next(q[0])
                    except StopIteration:
                        q.popleft()
                        continue
                    emit()

            # ---- attention blocks ----

            def scores(r, p, j):
                v = j - 4 * r
                off = 128 * v if v > 0 else 0
                s2 = ps_sc.tile([128, 1024], F32, tag="s2", bufs=2)
                qs = slice(512 * r + off, 512 * (r + 1))
                nc.tensor.matmul(
                    s2[:, off:512],
                    KT[0:64, p, 128 * j : 128 * (j + 1)],
                    QT[0:64, p, qs],
                    start=True, stop=True,
                )
                nc.tensor.matmul(
                    s2[:, 512 + off : 1024],
                    KT[64:128, p, 128 * j : 128 * (j + 1)],
                    QT[64:128, p, qs],
                    start=True, stop=True,
                )
                pt2 = ppool.tile([128, 1024], MMDT, tag="pt")
                if v >= 0:
                    s2v = s2[:, :].rearrange("p (s q) -> p s q", s=2)
                    pt2v = pt2[:, :].rearrange("p (s q) -> p s q", s=2)
                    nc.scalar.activation(
                        pt2v[:, :, off:512], s2v[:, :, off:512], EXPF, scale=SCALE
                    )
                    # zero the 128x128 upper triangle on the diagonal
                    nc.gpsimd.affine_select(
                        out=pt2v[:, :, off : off + 128],
                        in_=pt2v[:, :, off : off + 128],
                        compare_op=mybir.AluOpType.is_ge,
                        fill=0.0,
                        base=0,
                        pattern=[[0, 2], [1, 128]],
                        channel_multiplier=-1,
                    )
                else:
                    nc.scalar.activation(pt2[:], s2[:], EXPF, scale=SCALE)
                return pt2, off

            def pv(r, p, j, pt2, off, ca, cb, nj):
                hA, hB = 2 * p, 2 * p + 1
                nc.tensor.matmul(
                    ca[:, off:512], Vt[:, j, 65 * hA : 65 * hA + 65],
                    pt2[:, off:512],
                    start=(j == 0), stop=(j == nj - 1),
                )
                nc.tensor.matmul(
                    cb[:, off:512], Vt[:, j, 65 * hB : 65 * hB + 65],
                    pt2[:, 512 + off : 1024],
                    start=(j == 0), stop=(j == nj - 1),
                )

            norm_q = deque()

            def epilogue(r, p, ca, cb):
                # stage accumulators to SBUF immediately (frees the PSUM
                # banks for the next pair) and kick off the denominator
                # bounce; the rest of the normalization chain is deferred
                # one step per block so its cross-engine latency
                # (DMA -> recip -> gpsimd broadcast -> mul) never stalls the
                # in-order DVE/gpsimd queues that the attention pipe needs.
                # denominator rows hop from PSUM partition 64 straight to
                # SBUF partition 0 (32-aligned per-operand partition bases
                # are legal) BEFORE the bulk staging copies, so the
                # reciprocal chain starts ~1.4us earlier; then the ctx rows
                # are staged (also releasing the PSUM banks), and the rest
                # of the chain is deferred one step per block.
                srA = spool.tile([1, 512], F32, tag="sw")
                srB = spool.tile([1, 512], F32, tag="sw")
                nc.vector.tensor_copy(srA[:], ca[64:65, :])
                nc.vector.tensor_copy(srB[:], cb[64:65, :])
                stA = spool.tile([64, 512], F32, tag="st")
                stB = spool.tile([64, 512], F32, tag="st")
                nc.vector.tensor_copy(stA[:], ca[0:64, :])
                nc.vector.tensor_copy(stB[:], cb[0:64, :])
                ra = spool.tile([1, 512], F32, tag="rc")
                rb = spool.tile([1, 512], F32, tag="rc")
                bca = spool.tile([64, 512], F32, tag="bc")
                bcb = spool.tile([64, 512], F32, tag="bc")
                qs = slice(512 * r, 512 * (r + 1))
                norm_q.extend([
                    lambda: (
                        nc.vector.reciprocal_approx_fast(ra[:], srA[:]),
                        nc.vector.reciprocal_approx_fast(rb[:], srB[:]),
                    ),
                    lambda: nc.gpsimd.partition_broadcast(bca[:], ra[:]),
                    lambda: nc.vector.tensor_mul(
                        ctxT[0:64, p, qs], stA[:], bca[:]
                    ),
                    lambda: nc.gpsimd.partition_broadcast(bcb[:], rb[:]),
                    lambda: nc.vector.tensor_mul(
                        ctxT[64:128, p, qs], stB[:], bcb[:]
                    ),
                ])

            # ---- prologue ----
            # weight/x loads are split into multiple descriptors and spread
            # over four engines' dma queues: each DMA_DIRECT2D costs ~600ns
            # of issuing-engine time, so a single queue serializes the
            # startup. All engines are idle here anyway.
            wqv = wq.rearrange("p (c o) -> p c o", o=GW)
            wkv = wk.rearrange("p (c o) -> p c o", o=GW)
            wvv = wv.rearrange("p (c o) -> p c o", o=GW)
            wov = wo.rearrange("p (c o) -> p c o", o=D)
            # sync: wq chunks interleaved with the x chunks in the order the
            # first projection chain consumes them, then x1 (needed last)
            xt0 = xpool.tile([128, NC, 512], MMDT, tag="xt")
            xv0 = xT[:, 0:512].rearrange("(c p) t -> p c t", p=128)
            # gpsimd dispatches the first two x chunks in parallel with
            # sync's wq0, BEFORE the gpsimd library warm-load occupies it:
            # the first projection matmul is gated on chunk 0's last byte
            nc.gpsimd.dma_start(xt0[:, 0, :], xv0[:, 0, :])
            nc.gpsimd.dma_start(xt0[:, 1, :], xv0[:, 1, :])
            # warm up the gpsimd custom-op library (partition_broadcast
            # lives in a dynamically-loaded lib; the ~7us load runs here,
            # during the initial DMA wait, not at the first epilogue)
            nc.gpsimd.partition_broadcast(warmb[:], warm[:])
            nc.sync.dma_start(wq_sb[:, 0:1, :], wqv[:, 0:1, :])
            nc.sync.dma_start(wq_sb[:, 1:4, :], wqv[:, 1:4, :])
            for c in range(2, 4):
                nc.sync.dma_start(xt0[:, c, :], xv0[:, c, :])
            nc.sync.dma_start(wq_sb[:, 4:NC, :], wqv[:, 4:NC, :])
            for c in range(4, NC):
                nc.sync.dma_start(xt0[:, c, :], xv0[:, c, :])
            xts[0] = xt0
            # scalar: wk (gates the k chain), then wv (gates v chains), then
            # x1 -- sequenced behind the weights so its 1MB doesn't steal
            # DMA bandwidth from x0 during the startup-critical window
            for c in range(0, NC, 2):
                nc.scalar.dma_start(wk_sb[:, c : c + 2, :], wkv[:, c : c + 2, :])
            for c in range(0, NC, 2):
                nc.scalar.dma_start(wv_sb[:, c : c + 2, :], wvv[:, c : c + 2, :])
            load_xt(1)
            nc.gpsimd.dma_start(wo_sb[:, 0:1, :], wov[:, 0:1, :])
            nc.gpsimd.dma_start(wo_sb[:, 1:2, :], wov[:, 1:2, :])

            # emit only pair-0's q/k chains eagerly; the V chains drain
            # just-in-time before each PV(0,0,j), and the o=1 chains before
            # pair 1 -- so the first scores matmul and the exp pipeline
            # start ~5us earlier.
            for g in (qk_chain(0, wq_sb, QT, 0), qk_chain(0, wk_sb, KT, 0)):
                for _, emit in g:
                    emit()
            v_q.extend(v_chain(0, tt) for tt in range(4))
            kv_q.append(qk_chain(0, wq_sb, QT, 1))
            kv_q.append(qk_chain(0, wk_sb, KT, 1))

            def need_v0(j):
                # fully emit v_chain(0, 0..j) before PV(0,0,j)
                while vdone[0] <= j and v_q:
                    for _, emit in v_q[0]:
                        emit()
                    v_q.popleft()
                    vdone[0] += 1

            # ---- main loop: software-pipelined block stream ----
            kv_held = []
            v_held = []
            for r in range(NT):
                nj = 4 * r + 4
                kv_q.extend(kv_held)
                kv_held = []
                v_q.extend(v_held)
                v_held = []
                vdone[0] = 0
                if r + 1 < NT:
                    urgent_q.extend(q_chain_gens(r + 1))
                    kv_held = k_chain_gens(r + 1)
                    v_held = v_chain_gens(r + 1)
                else:
                    # keep several wo chains in reserve: they feed the
                    # exp-starved last range and cover the PE through the
                    # final pair's normalization latency
                    defer_floor[0] = 4
                if r + 2 < NT:
                    load_xt(r + 2)
                if r > 0:
                    defer_q.extend(c_chain_gens(r - 1))

                blocks = [(p, j) for p in (0, 1) for j in range(nj)]
                acc = {}
                pend = {}

                def emit_scores(b):
                    # K/V chunks 4r..4r+3 come from this range's deferred
                    # chains (for r=0, pair 1's Q/K are deferred); they must
                    # be fully emitted (in-order PE queue) before any scores
                    # that reads them.
                    if (r == 0 and b[0] == 1) or (r > 0 and b[1] >= 4 * r):
                        drain(kv_q)
                    pend[b] = scores(r, *b)

                emit_scores(blocks[0])
                for i, (p, j) in enumerate(blocks):
                    if i + 1 < len(blocks):
                        emit_scores(blocks[i + 1])
                    if norm_q:
                        norm_q.popleft()()
                    pop_filler(500)
                    if j == 0:
                        acc[p] = (
                            ps_acc.tile([65, 512], F32, tag="acc", bufs=2, name="ca"),
                            ps_acc.tile([65, 512], F32, tag="acc", bufs=2, name="cb"),
                        )
                    need_v0(j - 4 * r)
                    pt2, off = pend.pop((p, j))
                    pv(r, p, j, pt2, off, *acc[p], nj)
                    if j == nj - 1:
                        epilogue(r, p, *acc[p])
                need_v0(3)  # safety: all of this range's V chains emitted
                drain(kv_q)
                drain(urgent_q)
            # ---- tail: wo chains for the last range ----
            # PE cover for the last pair's normalization latency: first the
            # reserved wo chains of range NT-2 (fully ready), then the d=0
            # matmuls of the two s2-pool chains (depend only on pair-0 ctx).
            # The s2 banks are dead here, so these don't collide with the
            # norm broadcasts' mm-pool tiles.
            defer_floor[0] = 0
            drain(defer_q)
            tail = [
                wo_chain(NT - 1, qq, o, alt_pool=bool((2 * qq + o) % 2),
                         tail_idx=2 * qq + o)
                for qq in range(4) for o in range(2)
            ]
            pre = [g for g in tail if tail.index(g) % 2 == 1][:2]
            for g in pre:
                cost, emit = next(g)
                emit()
            while norm_q:
                norm_q.popleft()()
            defer_q.extend(tail)
            drain(defer_q)

    nc.compile()
    return nc


def _get_nc():
    if "nc" not in _CACHE:
        _CACHE["nc"] = _build()
    return _CACHE["nc"]


def kernel(x, Wq, Wk, Wv, Wo, bo):
    global LAST_RESULTS
    x = np.asarray(x, dtype=np.float32)
    Wq = np.asarray(Wq, dtype=np.float32)
    Wk = np.asarray(Wk, dtype=np.float32)
    Wv = np.asarray(Wv, dtype=np.float32)
    Wo = np.asarray(Wo, dtype=np.float32)
    bo = np.asarray(bo, dtype=np.float32)

    nc = _get_nc()
    xTs = [np.ascontiguousarray(x[b].T).astype(NPDT) for b in range(B)]

    def warr(w, cs):
        # [D, GW] slice -> [128, NC*GW]: partition p holds chunk-major rows
        s = w[:, cs].reshape(D // 128, 128, GW).transpose(1, 0, 2)
        return np.ascontiguousarray(s.reshape(128, -1)).astype(NPDT)

    def woarr(cs):
        # [GW, D] slice -> [128, 2*D]
        s = Wo[cs, :].reshape(GW // 128, 128, D).transpose(1, 0, 2)
        return np.ascontiguousarray(s.reshape(128, -1)).astype(NPDT)

    in_maps = []
    for c in range(N_CORES):
        b, g = divmod(c, N_CORES // B)
        cs = slice(GW * g, GW * (g + 1))
        in_maps.append(
            {
                "xT": xTs[b],
                "wq": warr(Wq, cs),
                "wk": warr(Wk, cs),
                "wv": warr(Wv, cs),
                "wo": woarr(cs),
            }
        )

    _maybe_install_trace_hook()
    res = bass_utils.run_bass_kernel_spmd(nc, in_maps, core_ids=list(range(N_CORES)))
    LAST_RESULTS = res

    out = np.zeros((B, S, D), dtype=np.float32)
    for c in range(N_CORES):
        out[c // (N_CORES // B)] += res.results[c]["out"]
    out += bo[None, None, :]
    return out


# revision 69
# speedup vs baseline: 1.0010x; 1.0010x over previous
"""Trainium2 Bass kernel for multi-head causal attention.

Problem: B=2, S=2048, D=1024, H=16 heads (head_dim=64), fp32.
  q,k,v = x@Wq, x@Wk, x@Wv  (per-head split)
  scores = q@k^T, causal mask, softmax(scores/sqrt(64))
  out = (attn@v concat) @ Wo + bo

Sharding (8 cores): core c -> batch b=c//4, head group g=c%4 (4 heads).
Each core computes its 4 heads' attention plus the partial output
projection (row-parallel Wo); host sums 4 partials per batch and adds bo.

Layout strategy (zero on-device transposes):
 - x^T passed host-transposed (feature-major).
 - Q^T,K^T produced feature-major: (head_dim x tokens), two heads stacked
   per 128-partition tile; scores^T computed per 64-partition row group
   (the PE runs the two 64-row matmuls concurrently in row groups).
 - Both heads' scores^T tiles (k x q) land in one 2-bank PSUM tile so the
   exp runs as a single wide instruction. The exp'd bf16 tile is directly
   the PV stationary operand. V is token-major with an appended ones
   column so the PV matmul also emits the softmax denominators.
 - causal structure: diagonal 128x512 blocks trim their scores / exp /
   PV widths to the valid q range; the 128x128 triangle on the diagonal
   is zeroed post-exp with one gpsimd affine_select (no mask adds, no
   memsets, nothing on the DVE or scalar engines).
 - scalar engine runs ONLY the exp chain mid-kernel; everything else
   (projection staging, accumulator staging, normalization, Wo staging)
   is on DVE/gpsimd/DMA so softmax never waits.
 - emission is a global software pipeline: scores for block i+1 and
   filler matmuls (QKV projections for range r+1, Wo for range r-1) are
   emitted between scores(i) and PV(i), so the in-order PE queue always
   has ~1us of exp-independent work in front of each PV and never stalls
   on the softmax chain. K/V chains for range r+1 are held back and
   drained inside range r+1 (before its diagonal blocks need them) so the
   late, exp-starved ranges still have PE filler. PSUM: 2x2 banks
   scores, 2 banks PV accum, 2 banks projections = 8; the dead scores
   banks are recycled for the tail Wo chains.
 - normalization: denominator row hops PSUM partition 64 -> SBUF
   partition 0 with one DVE copy (32-aligned cross-partition-base
   operands are legal), then fast reciprocal -> gpsimd partition
   broadcast -> multiply, deferred one step per block so the cross-engine
   latency never blocks the in-order queues. The gpsimd broadcast
   library is warm-loaded at t=0 (the ~7us load hides under the initial
   DMA wait).
 - startup: weight/x DMAs are split into many descriptors, spread over
   the sync/scalar/gpsimd queues in need-order (each descriptor costs
   ~600ns of issuing-engine time), and x range 1 is sequenced behind
   range 0 so it doesn't steal DMA bandwidth from the critical path.
Matmul operands are bf16 (1 cycle/row PE rate); every accumulation and
the softmax normalization stay fp32 in PSUM.
"""

import sys

sys.path.insert(0, "/opt/trn_rl_repo")

from collections import deque

import ml_dtypes
import numpy as np

import concourse.bass as bass  # noqa: F401
import concourse.tile as tile
from concourse import bacc, bass_utils, mybir

F32 = mybir.dt.float32
MMDT = mybir.dt.bfloat16
NPDT = ml_dtypes.bfloat16
EXPF = mybir.ActivationFunctionType.Exp

B, S, D, H, HD = 2, 2048, 1024, 16, 64
N_CORES = 8
HPC = 4            # heads per core
GW = HPC * HD      # head-group width per core = 256
SCALE = 1.0 / np.sqrt(HD)

_CACHE = {}
LAST_RESULTS = None


def _maybe_install_trace_hook():
    """If BASS_TRACE is set, bass_utils needs antenv.axon_hooks (absent in
    this image). Install it from trn_boot when possible; otherwise disable
    tracing so the run still works."""
    import os

    if not os.environ.get("BASS_TRACE"):
        return
    try:
        import antenv.axon_hooks  # noqa: F401
        return
    except ImportError:
        pass
    try:
        import types

        from trn_agent_boot.trn_boot import _ntff_profile_via_ctypes

        hook = _ntff_profile_via_ctypes("/opt/axon/libaxon_pjrt.so")
        mod = types.ModuleType("antenv.axon_hooks")
        mod.get_axon_ntff_profile_hook = lambda: hook
        mod.set_axon_ntff_profile_hook = lambda h: None
        import antenv

        sys.modules["antenv.axon_hooks"] = mod
        antenv.axon_hooks = mod
    except Exception:
        os.environ["BASS_NEVER_TRACE"] = "1"


def _build():
    nc = bacc.Bacc("TRN2", target_bir_lowering=False, debug=False)

    xT = nc.dram_tensor("xT", [D, S], MMDT, kind="ExternalInput").ap()
    wq = nc.dram_tensor("wq", [128, D // 128 * GW], MMDT, kind="ExternalInput").ap()
    wk = nc.dram_tensor("wk", [128, D // 128 * GW], MMDT, kind="ExternalInput").ap()
    wv = nc.dram_tensor("wv", [128, D // 128 * GW], MMDT, kind="ExternalInput").ap()
    wo = nc.dram_tensor("wo", [128, GW // 128 * D], MMDT, kind="ExternalInput").ap()
    out = nc.dram_tensor("out", [S, D], F32, kind="ExternalOutput").ap()

    NT = S // 512          # 4 q/t ranges of 512
    NC = D // 128          # 8 contraction chunks for projections
    NJ = S // 128          # 16 k-chunks

    with tile.TileContext(nc) as tc, nc.allow_low_precision(reason="bf16 matmuls"):
        with (
            tc.tile_pool(name="const", bufs=1) as cpool,
            tc.tile_pool(name="xin", bufs=3) as xpool,
            tc.tile_pool(name="pt", bufs=8) as ppool,
            tc.tile_pool(name="small", bufs=8) as spool,
            tc.tile_pool(name="ost", bufs=6) as opool,
            tc.tile_pool(name="ps_sc", bufs=1, space="PSUM") as ps_sc,
            tc.tile_pool(name="ps_acc", bufs=1, space="PSUM") as ps_acc,
            tc.tile_pool(name="ps_mm", bufs=1, space="PSUM") as ps_mm,
        ):
            # ---- persistent tiles ----
            wq_sb = cpool.tile([128, NC, GW], MMDT)
            wk_sb = cpool.tile([128, NC, GW], MMDT)
            wv_sb = cpool.tile([128, NC, GW], MMDT)
            wo_sb = cpool.tile([128, 2, D], MMDT)

            QT = cpool.tile([128, 2, S], MMDT)   # [:, pair, t] feature-major
            KT = cpool.tile([128, 2, S], MMDT)
            Vt = cpool.tile([128, NJ, HPC * 65], MMDT)  # token-major + ones col
            ctxT = cpool.tile([128, 2, S], MMDT)

            # ones columns of V (col 64 of each 65-wide head slot)
            vt_ones = Vt[:, :, :].rearrange("p j (h u) -> p (j h) u", u=65)[:, :, 64:65]
            nc.vector.memset(vt_ones, 1.0)
            warm = cpool.tile([1, 8], F32)
            nc.vector.memset(warm[:], 1.0)
            warmb = cpool.tile([2, 8], F32)

            xts = {}

            def load_xt(r, eng=None):
                xt = xpool.tile([128, NC, 512], MMDT, tag="xt")
                xv = xT[:, 512 * r : 512 * (r + 1)].rearrange("(c p) t -> p c t", p=128)
                for c in range(NC):
                    (eng or nc.sync).dma_start(xt[:, c, :], xv[:, c, :])
                xts[r] = xt

            # ---- projection / output chains as (cost, emit) generators ----

            def qk_chain(r, w_sb, dst, o):
                pm = ps_mm.tile([128, 512], F32, tag="mm", bufs=2)
                for c in range(NC):
                    yield 215, lambda c=c: nc.tensor.matmul(
                        pm[:],
                        w_sb[:, c, 128 * o : 128 * (o + 1)],
                        xts[r][:, c, :],
                        start=(c == 0),
                        stop=(c == NC - 1),
                    )
                yield 0, lambda: nc.vector.tensor_copy(
                    dst[:, o, 512 * r : 512 * (r + 1)], pm[:]
                )

            def v_chain(r, tt):
                j = 4 * r + tt
                pv = ps_mm.tile([128, 512], F32, tag="mm", bufs=2)
                for c in range(NC):
                    yield 110, lambda c=c: nc.tensor.matmul(
                        pv[:, 0:GW],
                        xts[r][:, c, 128 * tt : 128 * (tt + 1)],
                        wv_sb[:, c, :],
                        start=(c == 0),
                        stop=(c == NC - 1),
                    )
                yield 0, lambda: nc.vector.tensor_copy(
                    Vt[:, j, :].rearrange("p (h u) -> p h u", u=65)[:, :, 0:64],
                    pv[:, 0:GW].rearrange("p (h d) -> p h d", d=HD),
                )

            def wo_chain(r, qq, o, alt_pool=False, tail_idx=None):
                qt = 4 * r + qq
                if alt_pool:
                    # the scores banks are dead in the tail; reuse them so
                    # four chains pipeline instead of two
                    pot = ps_sc.tile([128, 1024], F32, tag="s2", bufs=2, name="po")
                    po = pot[:, 0:512]
                else:
                    pot = ps_mm.tile([128, 512], F32, tag="mm", bufs=2, name="po")
                    po = pot[:, :]
                for d in range(2):
                    yield 215, lambda d=d: nc.tensor.matmul(
                        po,
                        ctxT[:, d, 128 * qt : 128 * (qt + 1)],
                        wo_sb[:, d, 512 * o : 512 * (o + 1)],
                        start=(d == 0), stop=(d == 1),
                    )

                def fin():
                    ot = opool.tile([128, 512], F32, tag="ot")
                    # in the tail the scalar engine is done with exps and the
                    # DMA dispatch queues are idle: spread staging copies and
                    # output descriptors across engines so chains pipeline
                    if tail_idx is not None and tail_idx % 2 == 0:
                        nc.scalar.copy(ot[:], po)
                    else:
                        nc.vector.tensor_copy(ot[:], po)
                    eng = (
                        nc.sync if tail_idx is None
                        else (nc.scalar, nc.gpsimd, nc.sync)[tail_idx % 3]
                    )
                    eng.dma_start(
                        out[128 * qt : 128 * (qt + 1), 512 * o : 512 * (o + 1)],
                        ot[:],
                    )
                yield 0, fin

            def q_chain_gens(r):
                return [qk_chain(r, wq_sb, QT, o) for o in range(2)]

            def kv_chain_gens(r):
                ch = [qk_chain(r, wk_sb, KT, o) for o in range(2)]
                ch += [v_chain(r, tt) for tt in range(4)]
                return ch

            def c_chain_gens(r):
                return [wo_chain(r, qq, o) for qq in range(4) for o in range(2)]

            # three priorities; generators always run front-to-completion so
            # psum "mm" buffer rotation matches emission order.
            kv_q = deque()      # this range's K/V chains (most urgent)
            urgent_q = deque()  # next range's Q chains (drained by boundary)
            defer_q = deque()   # Wo chains for finished ranges (lazy)

            defer_floor = [0]
            v_q = deque()
            vdone = [0]

            def pop_filler(budget):
                while budget > 0:
                    if v_q:
                        try:
                            cost, emit = next(v_q[0])
                        except StopIteration:
                            v_q.popleft()
                            vdone[0] += 1
                            continue
                        emit()
                        budget -= cost
                        continue
                    q = kv_q or urgent_q
                    if not q:
                        # wo chains read ctxT: make sure every pending
                        # normalization step is emitted first
                        while norm_q:
                            norm_q.popleft()()
                        if len(defer_q) <= defer_floor[0]:
                            return
                        q = defer_q
                    if not q:
                        return
                    try:
                        cost, emit = next(q[0])
                    except StopIteration:
                        q.popleft()
                        continue
                    emit()
                    budget -= cost

            def drain(q):
                while q:
                    try:
                        cost, emit = next(q[0])
                    except StopIteration:
                        q.popleft()
                        continue
                    emit()

            # ---- attention blocks ----

            def scores(r, p, j):
                v = j - 4 * r
                off = 128 * v if v > 0 else 0
                s2 = ps_sc.tile([128, 1024], F32, tag="s2", bufs=2)
                qs = slice(512 * r + off, 512 * (r + 1))
                nc.tensor.matmul(
                    s2[:, off:512],
                    KT[0:64, p, 128 * j : 128 * (j + 1)],
                    QT[0:64, p, qs],
                    start=True, stop=True,
                )
                nc.tensor.matmul(
                    s2[:, 512 + off : 1024],
                    KT[64:128, p, 128 * j : 128 * (j + 1)],
                    QT[64:128, p, qs],
                    start=True, stop=True,
                )
                pt2 = ppool.tile([128, 1024], MMDT, tag="pt")
                if v >= 0:
                    s2v = s2[:, :].rearrange("p (s q) -> p s q", s=2)
                    pt2v = pt2[:, :].rearrange("p (s q) -> p s q", s=2)
                    nc.scalar.activation(
                        pt2v[:, :, off:512], s2v[:, :, off:512], EXPF, scale=SCALE
                    )
                    # zero the 128x128 upper triangle on the diagonal
                    nc.gpsimd.affine_select(
                        out=pt2v[:, :, off : off + 128],
                        in_=pt2v[:, :, off : off + 128],
                        compare_op=mybir.AluOpType.is_ge,
                        fill=0.0,
                        base=0,
                        pattern=[[0, 2], [1, 128]],
                        channel_multiplier=-1,
                    )
                else:
                    nc.scalar.activation(pt2[:], s2[:], EXPF, scale=SCALE)
                return pt2, off

            def pv(r, p, j, pt2, off, ca, cb, nj):
                hA, hB = 2 * p, 2 * p + 1
                nc.tensor.matmul(
                    ca[:, off:512], Vt[:, j, 65 * hA : 65 * hA + 65],
                    pt2[:, off:512],
                    start=(j == 0), stop=(j == nj - 1),
                )
                nc.tensor.matmul(
                    cb[:, off:512], Vt[:, j, 65 * hB : 65 * hB + 65],
                    pt2[:, 512 + off : 1024],
                    start=(j == 0), stop=(j == nj - 1),
                )

            norm_q = deque()

            def epilogue(r, p, ca, cb):
                # stage accumulators to SBUF immediately (frees the PSUM
                # banks for the next pair) and kick off the denominator
                # bounce; the rest of the normalization chain is deferred
                # one step per block so its cross-engine latency
                # (DMA -> recip -> gpsimd broadcast -> mul) never stalls the
                # in-order DVE/gpsimd queues that the attention pipe needs.
                stA = spool.tile([65, 512], F32, tag="st")
                stB = spool.tile([65, 512], F32, tag="st")
                nc.vector.tensor_copy(stA[:], ca[:])
                nc.vector.tensor_copy(stB[:], cb[:])
                # denominator rows hop from partition 64 to partition 0 with
                # a single DVE copy (32-aligned per-operand partition bases
                # are legal) -- no DMA bounce, so the chain has no dma-sem
                # latency; then fast reciprocal + gpsimd broadcast + multiply
                srA = spool.tile([1, 512], F32, tag="sw")
                srB = spool.tile([1, 512], F32, tag="sw")
                ra = spool.tile([1, 512], F32, tag="rc")
                rb = spool.tile([1, 512], F32, tag="rc")
                bca = spool.tile([64, 512], F32, tag="bc")
                bcb = spool.tile([64, 512], F32, tag="bc")
                qs = slice(512 * r, 512 * (r + 1))
                norm_q.extend([
                    lambda: (
                        nc.vector.tensor_copy(srA[:], stA[64:65, :]),
                        nc.vector.tensor_copy(srB[:], stB[64:65, :]),
                    ),
                    lambda: (
                        nc.vector.reciprocal_approx_fast(ra[:], srA[:]),
                        nc.vector.reciprocal_approx_fast(rb[:], srB[:]),
                    ),
                    lambda: nc.gpsimd.partition_broadcast(bca[:], ra[:]),
                    lambda: nc.vector.tensor_mul(
                        ctxT[0:64, p, qs], stA[0:64, :], bca[:]
                    ),
                    lambda: nc.gpsimd.partition_broadcast(bcb[:], rb[:]),
                    lambda: nc.vector.tensor_mul(
                        ctxT[64:128, p, qs], stB[0:64, :], bcb[:]
                    ),
                ])

            # ---- prologue ----
            # weight/x loads are split into multiple descriptors and spread
            # over four engines' dma queues: each DMA_DIRECT2D costs ~600ns
            # of issuing-engine time, so a single queue serializes the
            # startup. All engines are idle here anyway.
            wqv = wq.rearrange("p (c o) -> p c o", o=GW)
            wkv = wk.rearrange("p (c o) -> p c o", o=GW)
            wvv = wv.rearrange("p (c o) -> p c o", o=GW)
            wov = wo.rearrange("p (c o) -> p c o", o=D)
            # sync: wq chunks interleaved with the x chunks in the order the
            # first projection chain consumes them, then x1 (needed last)
            xt0 = xpool.tile([128, NC, 512], MMDT, tag="xt")
            xv0 = xT[:, 0:512].rearrange("(c p) t -> p c t", p=128)
            # gpsimd dispatches the first two x chunks in parallel with
            # sync's wq0, BEFORE the gpsimd library warm-load occupies it:
            # the first projection matmul is gated on chunk 0's last byte
            nc.gpsimd.dma_start(xt0[:, 0, :], xv0[:, 0, :])
            nc.gpsimd.dma_start(xt0[:, 1, :], xv0[:, 1, :])
            # warm up the gpsimd custom-op library (partition_broadcast
            # lives in a dynamically-loaded lib; the ~7us load runs here,
            # during the initial DMA wait, not at the first epilogue)
            nc.gpsimd.partition_broadcast(warmb[:], warm[:])
            nc.sync.dma_start(wq_sb[:, 0:1, :], wqv[:, 0:1, :])
            nc.sync.dma_start(wq_sb[:, 1:4, :], wqv[:, 1:4, :])
            for c in range(2, 4):
                nc.sync.dma_start(xt0[:, c, :], xv0[:, c, :])
            nc.sync.dma_start(wq_sb[:, 4:NC, :], wqv[:, 4:NC, :])
            for c in range(4, NC):
                nc.sync.dma_start(xt0[:, c, :], xv0[:, c, :])
            xts[0] = xt0
            # scalar: wk (gates the k chain), then wv (gates v chains), then
            # x1 -- sequenced behind the weights so its 1MB doesn't steal
            # DMA bandwidth from x0 during the startup-critical window
            for c in range(0, NC, 2):
                nc.scalar.dma_start(wk_sb[:, c : c + 2, :], wkv[:, c : c + 2, :])
            for c in range(0, NC, 2):
                nc.scalar.dma_start(wv_sb[:, c : c + 2, :], wvv[:, c : c + 2, :])
            load_xt(1)
            nc.gpsimd.dma_start(wo_sb[:, 0:1, :], wov[:, 0:1, :])
            nc.gpsimd.dma_start(wo_sb[:, 1:2, :], wov[:, 1:2, :])

            # emit only pair-0's q/k chains eagerly; the V chains drain
            # just-in-time before each PV(0,0,j), and the o=1 chains before
            # pair 1 -- so the first scores matmul and the exp pipeline
            # start ~5us earlier.
            for g in (qk_chain(0, wq_sb, QT, 0), qk_chain(0, wk_sb, KT, 0)):
                for _, emit in g:
                    emit()
            v_q.extend(v_chain(0, tt) for tt in range(4))
            kv_q.append(qk_chain(0, wq_sb, QT, 1))
            kv_q.append(qk_chain(0, wk_sb, KT, 1))

            def need_v0(j):
                # fully emit v_chain(0, 0..j) before PV(0,0,j)
                while vdone[0] <= j and v_q:
                    for _, emit in v_q[0]:
                        emit()
                    v_q.popleft()
                    vdone[0] += 1

            # ---- main loop: software-pipelined block stream ----
            kv_held = []
            for r in range(NT):
                nj = 4 * r + 4
                kv_q.extend(kv_held)
                kv_held = []
                if r + 1 < NT:
                    urgent_q.extend(q_chain_gens(r + 1))
                    kv_held = kv_chain_gens(r + 1)
                else:
                    # keep several wo chains in reserve: they feed the
                    # exp-starved last range and cover the PE through the
                    # final pair's normalization latency
                    defer_floor[0] = 4
                if r + 2 < NT:
                    load_xt(r + 2)
                if r > 0:
                    defer_q.extend(c_chain_gens(r - 1))

                blocks = [(p, j) for p in (0, 1) for j in range(nj)]
                acc = {}
                pend = {}

                def emit_scores(b):
                    # K/V chunks 4r..4r+3 come from this range's deferred
                    # chains (for r=0, pair 1's Q/K are deferred); they must
                    # be fully emitted (in-order PE queue) before any scores
                    # that reads them.
                    if (r == 0 and b[0] == 1) or (r > 0 and b[1] >= 4 * r):
                        drain(kv_q)
                    pend[b] = scores(r, *b)

                emit_scores(blocks[0])
                for i, (p, j) in enumerate(blocks):
                    if i + 1 < len(blocks):
                        emit_scores(blocks[i + 1])
                    if norm_q:
                        norm_q.popleft()()
                    pop_filler(500)
                    if j == 0:
                        acc[p] = (
                            ps_acc.tile([65, 512], F32, tag="acc", bufs=2, name="ca"),
                            ps_acc.tile([65, 512], F32, tag="acc", bufs=2, name="cb"),
                        )
                    if r == 0 and p == 0:
                        need_v0(j)
                    pt2, off = pend.pop((p, j))
                    pv(r, p, j, pt2, off, *acc[p], nj)
                    if j == nj - 1:
                        epilogue(r, p, *acc[p])
                drain(kv_q)
                drain(urgent_q)
            # ---- tail: wo chains for the last range ----
            # PE cover for the last pair's normalization latency: first the
            # reserved wo chains of range NT-2 (fully ready), then the d=0
            # matmuls of the two s2-pool chains (depend only on pair-0 ctx).
            # The s2 banks are dead here, so these don't collide with the
            # norm broadcasts' mm-pool tiles.
            defer_floor[0] = 0
            drain(defer_q)
            tail = [
                wo_chain(NT - 1, qq, o, alt_pool=bool((2 * qq + o) % 2),
                         tail_idx=2 * qq + o)
                for qq in range(4) for o in range(2)
            ]
            pre = [g for g in tail if tail.index(g) % 2 == 1][:2]
            for g in pre:
                cost, emit = next(g)
                emit()
            while norm_q:
                norm_q.popleft()()
            defer_q.extend(tail)
            drain(defer_q)

    nc.compile()
    return nc


def _get_nc():
    if "nc" not in _CACHE:
        _CACHE["nc"] = _build()
    return _CACHE["nc"]


def kernel(x, Wq, Wk, Wv, Wo, bo):
    global LAST_RESULTS
    x = np.asarray(x, dtype=np.float32)
    Wq = np.asarray(Wq, dtype=np.float32)
    Wk = np.asarray(Wk, dtype=np.float32)
    Wv = np.asarray(Wv, dtype=np.float32)
    Wo = np.asarray(Wo, dtype=np.float32)
    bo = np.asarray(bo, dtype=np.float32)

    nc = _get_nc()
    xTs = [np.ascontiguousarray(x[b].T).astype(NPDT) for b in range(B)]

    def warr(w, cs):
        # [D, GW] slice -> [128, NC*GW]: partition p holds chunk-major rows
        s = w[:, cs].reshape(D // 128, 128, GW).transpose(1, 0, 2)
        return np.ascontiguousarray(s.reshape(128, -1)).astype(NPDT)

    def woarr(cs):
        # [GW, D] slice -> [128, 2*D]
        s = Wo[cs, :].reshape(GW // 128, 128, D).transpose(1, 0, 2)
        return np.ascontiguousarray(s.reshape(128, -1)).astype(NPDT)

    in_maps = []
    for c in range(N_CORES):
        b, g = divmod(c, N_CORES // B)
        cs = slice(GW * g, GW * (g + 1))
        in_maps.append(
            {
                "xT": xTs[b],
                "wq": warr(Wq, cs),
                "wk": warr(Wk, cs),
                "wv": warr(Wv, cs),
                "wo": woarr(cs),
            }
        )

    _maybe_install_trace_hook()
    res = bass_utils.run_bass_kernel_spmd(nc, in_maps, core_ids=list(range(N_CORES)))
    LAST_RESULTS = res

    out = np.zeros((B, S, D), dtype=np.float32)
    for c in range(N_CORES):
        out[c // (N_CORES // B)] += res.results[c]["out"]
    out += bo[None, None, :]
    return out


# revision 72
# speedup vs baseline: 1.1594x; 1.1583x over previous
"""Trainium2 Bass kernel for multi-head causal attention.

Problem: B=2, S=2048, D=1024, H=16 heads (head_dim=64), fp32.
  q,k,v = x@Wq, x@Wk, x@Wv  (per-head split)
  scores = q@k^T, causal mask, softmax(scores/sqrt(64))
  out = (attn@v concat) @ Wo + bo

Sharding (8 cores): core c -> batch b=c//4, head group g=c%4 (4 heads).
Each core computes its 4 heads' attention plus the partial output
projection (row-parallel Wo); host sums 4 partials per batch and adds bo.

Layout strategy (zero on-device transposes):
 - x^T passed host-transposed (feature-major).
 - Q^T,K^T produced feature-major: (head_dim x tokens), two heads stacked
   per 128-partition tile; scores^T computed per 64-partition row group
   (the PE runs the two 64-row matmuls concurrently in row groups).
 - Both heads' scores^T tiles (k x q) land in one 2-bank PSUM tile so the
   exp runs as a single wide instruction. The exp'd bf16 tile is directly
   the PV stationary operand. V is token-major with an appended ones
   column so the PV matmul also emits the softmax denominators.
 - causal structure: diagonal 128x512 blocks trim their scores / exp /
   PV widths to the valid q range; the 128x128 triangle on the diagonal
   is zeroed post-exp with one gpsimd affine_select (no mask adds, no
   memsets, nothing on the DVE or scalar engines).
 - scalar engine runs ONLY the exp chain mid-kernel; everything else
   (projection staging, accumulator staging, normalization, Wo staging)
   is on DVE/gpsimd/DMA so softmax never waits.
 - emission is a global software pipeline: scores for block i+1 and
   filler matmuls (QKV projections for range r+1, Wo for range r-1) are
   emitted between scores(i) and PV(i), so the in-order PE queue always
   has ~1us of exp-independent work in front of each PV and never stalls
   on the softmax chain. K/V chains for range r+1 are held back and
   drained inside range r+1 (before its diagonal blocks need them) so the
   late, exp-starved ranges still have PE filler. PSUM: 2x2 banks
   scores, 2 banks PV accum, 2 banks projections = 8; the dead scores
   banks are recycled for the tail Wo chains.
 - normalization: denominator row hops PSUM partition 64 -> SBUF
   partition 0 with one DVE copy (32-aligned cross-partition-base
   operands are legal), then fast reciprocal -> gpsimd partition
   broadcast -> multiply, deferred one step per block so the cross-engine
   latency never blocks the in-order queues. The gpsimd broadcast
   library is warm-loaded at t=0 (the ~7us load hides under the initial
   DMA wait).
 - startup: weight/x DMAs are split into many descriptors, spread over
   the sync/scalar/gpsimd queues in need-order (each descriptor costs
   ~600ns of issuing-engine time), and x range 1 is sequenced behind
   range 0 so it doesn't steal DMA bandwidth from the critical path.
Matmul operands are bf16 (1 cycle/row PE rate); every accumulation and
the softmax normalization stay fp32 in PSUM.
"""

import sys

sys.path.insert(0, "/opt/trn_rl_repo")

from collections import deque

import ml_dtypes
import numpy as np

import concourse.bass as bass  # noqa: F401
import concourse.tile as tile
from concourse import bacc, bass_utils, mybir

F32 = mybir.dt.float32
MMDT = mybir.dt.bfloat16
NPDT = ml_dtypes.bfloat16
EXPF = mybir.ActivationFunctionType.Exp

B, S, D, H, HD = 2, 2048, 1024, 16, 64
N_CORES = 8
HPC = 4            # heads per core
GW = HPC * HD      # head-group width per core = 256
SCALE = 1.0 / np.sqrt(HD)

_CACHE = {}
LAST_RESULTS = None


def _maybe_install_trace_hook():
    """If BASS_TRACE is set, bass_utils needs antenv.axon_hooks (absent in
    this image). Install it from trn_boot when possible; otherwise disable
    tracing so the run still works."""
    import os

    if not os.environ.get("BASS_TRACE"):
        return
    try:
        import antenv.axon_hooks  # noqa: F401
        return
    except ImportError:
        pass
    try:
        import types

        from trn_agent_boot.trn_boot import _ntff_profile_via_ctypes

        hook = _ntff_profile_via_ctypes("/opt/axon/libaxon_pjrt.so")
        mod = types.ModuleType("antenv.axon_hooks")
        mod.get_axon_ntff_profile_hook = lambda: hook
        mod.set_axon_ntff_profile_hook = lambda h: None
        import antenv

        sys.modules["antenv.axon_hooks"] = mod
        antenv.axon_hooks = mod
    except Exception:
        os.environ["BASS_NEVER_TRACE"] = "1"


def _build():
    nc = bacc.Bacc("TRN2", target_bir_lowering=False, debug=False)

    xT = nc.dram_tensor("xT", [D, S], MMDT, kind="ExternalInput").ap()
    wq = nc.dram_tensor("wq", [128, D // 128 * GW], MMDT, kind="ExternalInput").ap()
    wk = nc.dram_tensor("wk", [128, D // 128 * GW], MMDT, kind="ExternalInput").ap()
    wv = nc.dram_tensor("wv", [128, D // 128 * GW], MMDT, kind="ExternalInput").ap()
    wo = nc.dram_tensor("wo", [128, GW // 128 * D], MMDT, kind="ExternalInput").ap()
    out = nc.dram_tensor("out", [S, D], F32, kind="ExternalOutput").ap()

    NT = S // 512          # 4 q/t ranges of 512
    NC = D // 128          # 8 contraction chunks for projections
    NJ = S // 128          # 16 k-chunks

    with tile.TileContext(nc) as tc, nc.allow_low_precision(reason="bf16 matmuls"):
        with (
            tc.tile_pool(name="const", bufs=1) as cpool,
            tc.tile_pool(name="xin", bufs=3) as xpool,
            tc.tile_pool(name="pt", bufs=8) as ppool,
            tc.tile_pool(name="small", bufs=8) as spool,
            tc.tile_pool(name="ost", bufs=6) as opool,
            tc.tile_pool(name="ps_sc", bufs=1, space="PSUM") as ps_sc,
            tc.tile_pool(name="ps_acc", bufs=1, space="PSUM") as ps_acc,
            tc.tile_pool(name="ps_mm", bufs=1, space="PSUM") as ps_mm,
        ):
            # ---- persistent tiles ----
            wq_sb = cpool.tile([128, NC, GW], MMDT)
            wk_sb = cpool.tile([128, NC, GW], MMDT)
            wv_sb = cpool.tile([128, NC, GW], MMDT)
            wo_sb = cpool.tile([128, 2, D], MMDT)

            QT = cpool.tile([128, 2, S], MMDT)   # [:, pair, t] feature-major
            KT = cpool.tile([128, 2, S], MMDT)
            Vt = cpool.tile([128, NJ, HPC * 65], MMDT)  # token-major + ones col
            ctxT = cpool.tile([128, 2, S], MMDT)

            # ones columns of V (col 64 of each 65-wide head slot)
            vt_ones = Vt[:, :, :].rearrange("p j (h u) -> p (j h) u", u=65)[:, :, 64:65]
            nc.vector.memset(vt_ones, 1.0)
            warm = cpool.tile([1, 8], F32)
            nc.vector.memset(warm[:], 1.0)
            warmb = cpool.tile([2, 8], F32)
            # triangular causal mask (keep q-k>=0, else -1e30) for the last
            # range's pre-exp masking; broadcast view over the 2 head slots
            tri = cpool.tile([128, 128], F32, name="tri")
            tri_ap = tri[:]
            tri2 = bass.AP(
                tensor=tri_ap.tensor,
                offset=tri_ap.offset,
                ap=[list(tri_ap.ap[0]), [0, 2], list(tri_ap.ap[1])],
            )

            xts = {}

            def load_xt(r, eng=None):
                xt = xpool.tile([128, NC, 512], MMDT, tag="xt")
                xv = xT[:, 512 * r : 512 * (r + 1)].rearrange("(c p) t -> p c t", p=128)
                for c in range(NC):
                    (eng or nc.sync).dma_start(xt[:, c, :], xv[:, c, :])
                xts[r] = xt

            # ---- projection / output chains as (cost, emit) generators ----

            def qk_chain(r, w_sb, dst, o):
                pm = ps_mm.tile([128, 512], F32, tag="mm", bufs=2)
                for c in range(NC):
                    yield 215, lambda c=c: nc.tensor.matmul(
                        pm[:],
                        w_sb[:, c, 128 * o : 128 * (o + 1)],
                        xts[r][:, c, :],
                        start=(c == 0),
                        stop=(c == NC - 1),
                    )
                yield 0, lambda: nc.vector.tensor_copy(
                    dst[:, o, 512 * r : 512 * (r + 1)], pm[:]
                )

            def v_chain(r, tt):
                j = 4 * r + tt
                pv = ps_mm.tile([128, 512], F32, tag="mm", bufs=2)
                for c in range(NC):
                    yield 110, lambda c=c: nc.tensor.matmul(
                        pv[:, 0:GW],
                        xts[r][:, c, 128 * tt : 128 * (tt + 1)],
                        wv_sb[:, c, :],
                        start=(c == 0),
                        stop=(c == NC - 1),
                    )
                yield 0, lambda: nc.vector.tensor_copy(
                    Vt[:, j, :].rearrange("p (h u) -> p h u", u=65)[:, :, 0:64],
                    pv[:, 0:GW].rearrange("p (h d) -> p h d", d=HD),
                )

            def wo_chain(r, qq, o, alt_pool=False, tail_idx=None):
                qt = 4 * r + qq
                if alt_pool:
                    # the scores banks are dead in the tail; reuse them so
                    # four chains pipeline instead of two
                    pot = ps_sc.tile([128, 1024], F32, tag="s2", bufs=2, name="po")
                    po = pot[:, 0:512]
                else:
                    pot = ps_mm.tile([128, 512], F32, tag="mm", bufs=2, name="po")
                    po = pot[:, :]
                for d in range(2):
                    yield 215, lambda d=d: nc.tensor.matmul(
                        po,
                        ctxT[:, d, 128 * qt : 128 * (qt + 1)],
                        wo_sb[:, d, 512 * o : 512 * (o + 1)],
                        start=(d == 0), stop=(d == 1),
                    )

                def fin():
                    ot = opool.tile([128, 512], F32, tag="ot")
                    # in the tail the scalar engine is done with exps and the
                    # DMA dispatch queues are idle: spread staging copies and
                    # output descriptors across engines so chains pipeline
                    if tail_idx is not None and tail_idx % 2 == 0:
                        nc.scalar.copy(ot[:], po)
                    else:
                        nc.vector.tensor_copy(ot[:], po)
                    eng = (
                        nc.sync if tail_idx is None
                        else (nc.scalar, nc.gpsimd, nc.sync)[tail_idx % 3]
                    )
                    eng.dma_start(
                        out[128 * qt : 128 * (qt + 1), 512 * o : 512 * (o + 1)],
                        ot[:],
                    )
                yield 0, fin

            def q_chain_gens(r):
                return [qk_chain(r, wq_sb, QT, o) for o in range(2)]

            def kv_chain_gens(r):
                ch = [qk_chain(r, wk_sb, KT, o) for o in range(2)]
                ch += [v_chain(r, tt) for tt in range(4)]
                return ch

            def c_chain_gens(r):
                return [wo_chain(r, qq, o) for qq in range(4) for o in range(2)]

            # three priorities; generators always run front-to-completion so
            # psum "mm" buffer rotation matches emission order.
            kv_q = deque()      # this range's K/V chains (most urgent)
            urgent_q = deque()  # next range's Q chains (drained by boundary)
            defer_q = deque()   # Wo chains for finished ranges (lazy)

            defer_floor = [0]
            v_q = deque()
            vdone = [0]

            def pop_filler(budget):
                while budget > 0:
                    if v_q:
                        try:
                            cost, emit = next(v_q[0])
                        except StopIteration:
                            v_q.popleft()
                            vdone[0] += 1
                            continue
                        emit()
                        budget -= cost
                        continue
                    q = kv_q or urgent_q
                    if not q:
                        # wo chains read ctxT: make sure every pending
                        # normalization step is emitted first
                        while norm_q:
                            norm_q.popleft()()
                        if len(defer_q) <= defer_floor[0]:
                            return
                        q = defer_q
                    if not q:
                        return
                    try:
                        cost, emit = next(q[0])
                    except StopIteration:
                        q.popleft()
                        continue
                    emit()
                    budget -= cost

            def drain(q):
                while q:
                    try:
                        cost, emit = next(q[0])
                    except StopIteration:
                        q.popleft()
                        continue
                    emit()

            # ---- attention blocks ----

            def scores(r, p, j):
                v = j - 4 * r
                off = 128 * v if v > 0 else 0
                s2 = ps_sc.tile([128, 1024], F32, tag="s2", bufs=2)
                qs = slice(512 * r + off, 512 * (r + 1))
                nc.tensor.matmul(
                    s2[:, off:512],
                    KT[0:64, p, 128 * j : 128 * (j + 1)],
                    QT[0:64, p, qs],
                    start=True, stop=True,
                )
                nc.tensor.matmul(
                    s2[:, 512 + off : 1024],
                    KT[64:128, p, 128 * j : 128 * (j + 1)],
                    QT[64:128, p, qs],
                    start=True, stop=True,
                )
                pt2 = ppool.tile([128, 1024], MMDT, tag="pt")
                if v >= 0:
                    s2v = s2[:, :].rearrange("p (s q) -> p s q", s=2)
                    pt2v = pt2[:, :].rearrange("p (s q) -> p s q", s=2)
                    if r == NT - 1:
                        # exp-starved last range: mask BEFORE exp with a DVE
                        # add (the DVE is idle here, and the add hides behind
                        # the previous block's exp) so the post-exp gpsimd
                        # select drops off the exp->PV pacing chain
                        nc.vector.tensor_add(
                            s2v[:, :, off : off + 128],
                            s2v[:, :, off : off + 128],
                            tri2,
                        )
                        nc.scalar.activation(
                            pt2v[:, :, off:512], s2v[:, :, off:512],
                            EXPF, scale=SCALE,
                        )
                    else:
                        nc.scalar.activation(
                            pt2v[:, :, off:512], s2v[:, :, off:512],
                            EXPF, scale=SCALE,
                        )
                        # zero the 128x128 upper triangle on the diagonal
                        nc.gpsimd.affine_select(
                            out=pt2v[:, :, off : off + 128],
                            in_=pt2v[:, :, off : off + 128],
                            compare_op=mybir.AluOpType.is_ge,
                            fill=0.0,
                            base=0,
                            pattern=[[0, 2], [1, 128]],
                            channel_multiplier=-1,
                        )
                else:
                    nc.scalar.activation(pt2[:], s2[:], EXPF, scale=SCALE)
                return pt2, off

            def pv(r, p, j, pt2, off, ca, cb, nj):
                hA, hB = 2 * p, 2 * p + 1
                nc.tensor.matmul(
                    ca[:, off:512], Vt[:, j, 65 * hA : 65 * hA + 65],
                    pt2[:, off:512],
                    start=(j == 0), stop=(j == nj - 1),
                )
                nc.tensor.matmul(
                    cb[:, off:512], Vt[:, j, 65 * hB : 65 * hB + 65],
                    pt2[:, 512 + off : 1024],
                    start=(j == 0), stop=(j == nj - 1),
                )

            norm_q = deque()

            def epilogue(r, p, ca, cb):
                # stage accumulators to SBUF immediately (frees the PSUM
                # banks for the next pair) and kick off the denominator
                # bounce; the rest of the normalization chain is deferred
                # one step per block so its cross-engine latency
                # (DMA -> recip -> gpsimd broadcast -> mul) never stalls the
                # in-order DVE/gpsimd queues that the attention pipe needs.
                stA = spool.tile([65, 512], F32, tag="st")
                stB = spool.tile([65, 512], F32, tag="st")
                nc.vector.tensor_copy(stA[:], ca[:])
                nc.vector.tensor_copy(stB[:], cb[:])
                # denominator rows hop from partition 64 to partition 0 with
                # a single DVE copy (32-aligned per-operand partition bases
                # are legal) -- no DMA bounce, so the chain has no dma-sem
                # latency; then fast reciprocal + gpsimd broadcast + multiply
                srA = spool.tile([1, 512], F32, tag="sw")
                srB = spool.tile([1, 512], F32, tag="sw")
                ra = spool.tile([1, 512], F32, tag="rc")
                rb = spool.tile([1, 512], F32, tag="rc")
                bca = spool.tile([64, 512], F32, tag="bc")
                bcb = spool.tile([64, 512], F32, tag="bc")
                qs = slice(512 * r, 512 * (r + 1))
                norm_q.extend([
                    lambda: (
                        nc.vector.tensor_copy(srA[:], stA[64:65, :]),
                        nc.vector.tensor_copy(srB[:], stB[64:65, :]),
                    ),
                    lambda: (
                        nc.vector.reciprocal_approx_fast(ra[:], srA[:]),
                        nc.vector.reciprocal_approx_fast(rb[:], srB[:]),
                    ),
                    lambda: nc.gpsimd.partition_broadcast(bca[:], ra[:]),
                    lambda: nc.vector.tensor_mul(
                        ctxT[0:64, p, qs], stA[0:64, :], bca[:]
                    ),
                    lambda: nc.gpsimd.partition_broadcast(bcb[:], rb[:]),
                    lambda: nc.vector.tensor_mul(
                        ctxT[64:128, p, qs], stB[0:64, :], bcb[:]
                    ),
                ])

            # ---- prologue ----
            # weight/x loads are split into multiple descriptors and spread
            # over four engines' dma queues: each DMA_DIRECT2D costs ~600ns
            # of issuing-engine time, so a single queue serializes the
            # startup. All engines are idle here anyway.
            wqv = wq.rearrange("p (c o) -> p c o", o=GW)
            wkv = wk.rearrange("p (c o) -> p c o", o=GW)
            wvv = wv.rearrange("p (c o) -> p c o", o=GW)
            wov = wo.rearrange("p (c o) -> p c o", o=D)
            # sync: wq chunks interleaved with the x chunks in the order the
            # first projection chain consumes them, then x1 (needed last)
            xt0 = xpool.tile([128, NC, 512], MMDT, tag="xt")
            xv0 = xT[:, 0:512].rearrange("(c p) t -> p c t", p=128)
            # gpsimd dispatches the first two x chunks in parallel with
            # sync's wq0, BEFORE the gpsimd library warm-load occupies it:
            # the first projection matmul is gated on chunk 0's last byte
            nc.gpsimd.dma_start(xt0[:, 0, :], xv0[:, 0, :])
            nc.gpsimd.dma_start(xt0[:, 1, :], xv0[:, 1, :])
            # warm up the gpsimd custom-op library (partition_broadcast
            # lives in a dynamically-loaded lib; the ~7us load runs here,
            # during the initial DMA wait, not at the first epilogue)
            nc.gpsimd.partition_broadcast(warmb[:], warm[:])
            nc.gpsimd.memset(tri[:], 0.0)
            nc.gpsimd.affine_select(
                out=tri[:],
                in_=tri[:],
                compare_op=mybir.AluOpType.is_ge,
                fill=-1.0e30,
                base=0,
                pattern=[[1, 128]],
                channel_multiplier=-1,
            )
            nc.sync.dma_start(wq_sb[:, 0:1, :], wqv[:, 0:1, :])
            nc.sync.dma_start(wq_sb[:, 1:4, :], wqv[:, 1:4, :])
            for c in range(2, 4):
                nc.sync.dma_start(xt0[:, c, :], xv0[:, c, :])
            nc.sync.dma_start(wq_sb[:, 4:NC, :], wqv[:, 4:NC, :])
            for c in range(4, NC):
                nc.sync.dma_start(xt0[:, c, :], xv0[:, c, :])
            xts[0] = xt0
            # scalar: wk (gates the k chain), then wv (gates v chains), then
            # x1 -- sequenced behind the weights so its 1MB doesn't steal
            # DMA bandwidth from x0 during the startup-critical window
            for c in range(0, NC, 2):
                nc.scalar.dma_start(wk_sb[:, c : c + 2, :], wkv[:, c : c + 2, :])
            for c in range(0, NC, 2):
                nc.scalar.dma_start(wv_sb[:, c : c + 2, :], wvv[:, c : c + 2, :])
            load_xt(1)
            nc.gpsimd.dma_start(wo_sb[:, 0:1, :], wov[:, 0:1, :])
            nc.gpsimd.dma_start(wo_sb[:, 1:2, :], wov[:, 1:2, :])

            # emit only pair-0's q/k chains eagerly; the V chains drain
            # just-in-time before each PV(0,0,j), and the o=1 chains before
            # pair 1 -- so the first scores matmul and the exp pipeline
            # start ~5us earlier.
            for g in (qk_chain(0, wq_sb, QT, 0), qk_chain(0, wk_sb, KT, 0)):
                for _, emit in g:
                    emit()
            v_q.extend(v_chain(0, tt) for tt in range(4))
            kv_q.append(qk_chain(0, wq_sb, QT, 1))
            kv_q.append(qk_chain(0, wk_sb, KT, 1))

            def need_v0(j):
                # fully emit v_chain(0, 0..j) before PV(0,0,j)
                while vdone[0] <= j and v_q:
                    for _, emit in v_q[0]:
                        emit()
                    v_q.popleft()
                    vdone[0] += 1

            # ---- main loop: software-pipelined block stream ----
            kv_held = []
            for r in range(NT):
                nj = 4 * r + 4
                kv_q.extend(kv_held)
                kv_held = []
                if r + 1 < NT:
                    urgent_q.extend(q_chain_gens(r + 1))
                    kv_held = kv_chain_gens(r + 1)
                else:
                    # keep several wo chains in reserve: they feed the
                    # exp-starved last range and cover the PE through the
                    # final pair's normalization latency
                    defer_floor[0] = 4
                if r + 2 < NT:
                    load_xt(r + 2)
                if r > 0:
                    defer_q.extend(c_chain_gens(r - 1))

                blocks = [(p, j) for p in (0, 1) for j in range(nj)]
                acc = {}
                pend = {}

                def emit_scores(b):
                    # K/V chunks 4r..4r+3 come from this range's deferred
                    # chains (for r=0, pair 1's Q/K are deferred); they must
                    # be fully emitted (in-order PE queue) before any scores
                    # that reads them.
                    if (r == 0 and b[0] == 1) or (r > 0 and b[1] >= 4 * r):
                        drain(kv_q)
                    pend[b] = scores(r, *b)

                emit_scores(blocks[0])
                for i, (p, j) in enumerate(blocks):
                    if i + 1 < len(blocks):
                        emit_scores(blocks[i + 1])
                    if norm_q:
                        norm_q.popleft()()
                    pop_filler(500)
                    if j == 0:
                        acc[p] = (
                            ps_acc.tile([65, 512], F32, tag="acc", bufs=2, name="ca"),
                            ps_acc.tile([65, 512], F32, tag="acc", bufs=2, name="cb"),
                        )
                    if r == 0 and p == 0:
                        need_v0(j)
                    pt2, off = pend.pop((p, j))
                    pv(r, p, j, pt2, off, *acc[p], nj)
                    if j == nj - 1:
                        epilogue(r, p, *acc[p])
                drain(kv_q)
                drain(urgent_q)
            # ---- tail: wo chains for the last range ----
            # PE cover for the last pair's normalization latency: first the
            # reserved wo chains of range NT-2 (fully ready), then the d=0
            # matmuls of the two s2-pool chains (depend only on pair-0 ctx).
            # The s2 banks are dead here, so these don't collide with the
            # norm broadcasts' mm-pool tiles.
            defer_floor[0] = 0
            drain(defer_q)
            tail = [
                wo_chain(NT - 1, qq, o, alt_pool=bool((2 * qq + o) % 2),
                         tail_idx=2 * qq + o)
                for qq in range(4) for o in range(2)
            ]
            pre = [g for g in tail if tail.index(g) % 2 == 1][:2]
            for g in pre:
                cost, emit = next(g)
                emit()
            while norm_q:
                norm_q.popleft()()
            defer_q.extend(tail)
            drain(defer_q)

    nc.compile()
    return nc


def _get_nc():
    if "nc" not in _CACHE:
        _CACHE["nc"] = _build()
    return _CACHE["nc"]


def kernel(x, Wq, Wk, Wv, Wo, bo):
    global LAST_RESULTS
    x = np.asarray(x, dtype=np.float32)
    Wq = np.asarray(Wq, dtype=np.float32)
    Wk = np.asarray(Wk, dtype=np.float32)
    Wv = np.asarray(Wv, dtype=np.float32)
    Wo = np.asarray(Wo, dtype=np.float32)
    bo = np.asarray(bo, dtype=np.float32)

    nc = _get_nc()
    xTs = [np.ascontiguousarray(x[b].T).astype(NPDT) for b in range(B)]

    def warr(w, cs):
        # [D, GW] slice -> [128, NC*GW]: partition p holds chunk-major rows
        s = w[:, cs].reshape(D // 128, 128, GW).transpose(1, 0, 2)
        return np.ascontiguousarray(s.reshape(128, -1)).astype(NPDT)

    def woarr(cs):
        # [GW, D] slice -> [128, 2*D]
        s = Wo[cs, :].reshape(GW // 128, 128, D).transpose(1, 0, 2)
        return np.ascontiguousarray(s.reshape(128, -1)).astype(NPDT)

    in_maps = []
    for c in range(N_CORES):
        b, g = divmod(c, N_CORES // B)
        cs = slice(GW * g, GW * (g + 1))
        in_maps.append(
            {
                "xT": xTs[b],
                "wq": warr(Wq, cs),
                "wk": warr(Wk, cs),
                "wv": warr(Wv, cs),
                "wo": woarr(cs),
            }
        )

    _maybe_install_trace_hook()
    res = bass_utils.run_bass_kernel_spmd(nc, in_maps, core_ids=list(range(N_CORES)))
    LAST_RESULTS = res

    out = np.zeros((B, S, D), dtype=np.float32)
    for c in range(N_CORES):
        out[c // (N_CORES // B)] += res.results[c]["out"]
    out += bo[None, None, :]
    return out


# revision 73
# speedup vs baseline: 1.1784x; 1.0164x over previous
"""Trainium2 Bass kernel for multi-head causal attention.

Problem: B=2, S=2048, D=1024, H=16 heads (head_dim=64), fp32.
  q,k,v = x@Wq, x@Wk, x@Wv  (per-head split)
  scores = q@k^T, causal mask, softmax(scores/sqrt(64))
  out = (attn@v concat) @ Wo + bo

Sharding (8 cores): core c -> batch b=c//4, head group g=c%4 (4 heads).
Each core computes its 4 heads' attention plus the partial output
projection (row-parallel Wo); host sums 4 partials per batch and adds bo.

Layout strategy (zero on-device transposes):
 - x^T passed host-transposed (feature-major).
 - Q^T,K^T produced feature-major: (head_dim x tokens), two heads stacked
   per 128-partition tile; scores^T computed per 64-partition row group
   (the PE runs the two 64-row matmuls concurrently in row groups).
 - Both heads' scores^T tiles (k x q) land in one 2-bank PSUM tile so the
   exp runs as a single wide instruction. The exp'd bf16 tile is directly
   the PV stationary operand. V is token-major with an appended ones
   column so the PV matmul also emits the softmax denominators.
 - causal structure: diagonal 128x512 blocks trim their scores / exp /
   PV widths to the valid q range; the 128x128 triangle on the diagonal
   is zeroed post-exp with one gpsimd affine_select (no mask adds, no
   memsets, nothing on the DVE or scalar engines).
 - scalar engine runs ONLY the exp chain mid-kernel; everything else
   (projection staging, accumulator staging, normalization, Wo staging)
   is on DVE/gpsimd/DMA so softmax never waits.
 - emission is a global software pipeline: scores for block i+1 and
   filler matmuls (QKV projections for range r+1, Wo for range r-1) are
   emitted between scores(i) and PV(i), so the in-order PE queue always
   has ~1us of exp-independent work in front of each PV and never stalls
   on the softmax chain. K/V chains for range r+1 are held back and
   drained inside range r+1 (before its diagonal blocks need them) so the
   late, exp-starved ranges still have PE filler. PSUM: 2x2 banks
   scores, 2 banks PV accum, 2 banks projections = 8; the dead scores
   banks are recycled for the tail Wo chains.
 - normalization: denominator row hops PSUM partition 64 -> SBUF
   partition 0 with one DVE copy (32-aligned cross-partition-base
   operands are legal), then fast reciprocal -> gpsimd partition
   broadcast -> multiply, deferred one step per block so the cross-engine
   latency never blocks the in-order queues. The gpsimd broadcast
   library is warm-loaded at t=0 (the ~7us load hides under the initial
   DMA wait).
 - startup: weight/x DMAs are split into many descriptors, spread over
   the sync/scalar/gpsimd queues in need-order (each descriptor costs
   ~600ns of issuing-engine time), and x range 1 is sequenced behind
   range 0 so it doesn't steal DMA bandwidth from the critical path.
Matmul operands are bf16 (1 cycle/row PE rate); every accumulation and
the softmax normalization stay fp32 in PSUM.
"""

import sys

sys.path.insert(0, "/opt/trn_rl_repo")

from collections import deque

import ml_dtypes
import numpy as np

import concourse.bass as bass  # noqa: F401
import concourse.tile as tile
from concourse import bacc, bass_utils, mybir

F32 = mybir.dt.float32
MMDT = mybir.dt.bfloat16
NPDT = ml_dtypes.bfloat16
EXPF = mybir.ActivationFunctionType.Exp

B, S, D, H, HD = 2, 2048, 1024, 16, 64
N_CORES = 8
HPC = 4            # heads per core
GW = HPC * HD      # head-group width per core = 256
SCALE = 1.0 / np.sqrt(HD)

_CACHE = {}
LAST_RESULTS = None


def _maybe_install_trace_hook():
    """If BASS_TRACE is set, bass_utils needs antenv.axon_hooks (absent in
    this image). Install it from trn_boot when possible; otherwise disable
    tracing so the run still works."""
    import os

    if not os.environ.get("BASS_TRACE"):
        return
    try:
        import antenv.axon_hooks  # noqa: F401
        return
    except ImportError:
        pass
    try:
        import types

        from trn_agent_boot.trn_boot import _ntff_profile_via_ctypes

        hook = _ntff_profile_via_ctypes("/opt/axon/libaxon_pjrt.so")
        mod = types.ModuleType("antenv.axon_hooks")
        mod.get_axon_ntff_profile_hook = lambda: hook
        mod.set_axon_ntff_profile_hook = lambda h: None
        import antenv

        sys.modules["antenv.axon_hooks"] = mod
        antenv.axon_hooks = mod
    except Exception:
        os.environ["BASS_NEVER_TRACE"] = "1"


def _build():
    nc = bacc.Bacc("TRN2", target_bir_lowering=False, debug=False)

    xT = nc.dram_tensor("xT", [D, S], MMDT, kind="ExternalInput").ap()
    wq = nc.dram_tensor("wq", [128, D // 128 * GW], MMDT, kind="ExternalInput").ap()
    wk = nc.dram_tensor("wk", [128, D // 128 * GW], MMDT, kind="ExternalInput").ap()
    wv = nc.dram_tensor("wv", [128, D // 128 * GW], MMDT, kind="ExternalInput").ap()
    wo = nc.dram_tensor("wo", [128, GW // 128 * D], MMDT, kind="ExternalInput").ap()
    out = nc.dram_tensor("out", [S, D], F32, kind="ExternalOutput").ap()

    NT = S // 512          # 4 q/t ranges of 512
    NC = D // 128          # 8 contraction chunks for projections
    NJ = S // 128          # 16 k-chunks

    with tile.TileContext(nc) as tc, nc.allow_low_precision(reason="bf16 matmuls"):
        with (
            tc.tile_pool(name="const", bufs=1) as cpool,
            tc.tile_pool(name="xin", bufs=3) as xpool,
            tc.tile_pool(name="pt", bufs=8) as ppool,
            tc.tile_pool(name="small", bufs=8) as spool,
            tc.tile_pool(name="ost", bufs=6) as opool,
            tc.tile_pool(name="ps_sc", bufs=1, space="PSUM") as ps_sc,
            tc.tile_pool(name="ps_acc", bufs=1, space="PSUM") as ps_acc,
            tc.tile_pool(name="ps_mm", bufs=1, space="PSUM") as ps_mm,
        ):
            # ---- persistent tiles ----
            wq_sb = cpool.tile([128, NC, GW], MMDT)
            wk_sb = cpool.tile([128, NC, GW], MMDT)
            wv_sb = cpool.tile([128, NC, GW], MMDT)
            wo_sb = cpool.tile([128, 2, D], MMDT)

            QT = cpool.tile([128, 2, S], MMDT)   # [:, pair, t] feature-major
            KT = cpool.tile([128, 2, S], MMDT)
            Vt = cpool.tile([128, NJ, HPC * 65], MMDT)  # token-major + ones col
            ctxT = cpool.tile([128, 2, S], MMDT)

            # ones columns of V (col 64 of each 65-wide head slot)
            vt_ones = Vt[:, :, :].rearrange("p j (h u) -> p (j h) u", u=65)[:, :, 64:65]
            nc.vector.memset(vt_ones, 1.0)
            warm = cpool.tile([1, 8], F32)
            nc.vector.memset(warm[:], 1.0)
            warmb = cpool.tile([2, 8], F32)
            # triangular causal mask (keep q-k>=0, else -1e30) for the last
            # range's pre-exp masking; broadcast view over the 2 head slots
            tri = cpool.tile([128, 128], F32, name="tri")
            tri_ap = tri[:]
            tri2 = bass.AP(
                tensor=tri_ap.tensor,
                offset=tri_ap.offset,
                ap=[list(tri_ap.ap[0]), [0, 2], list(tri_ap.ap[1])],
            )

            xts = {}

            def load_xt(r, eng=None):
                xt = xpool.tile([128, NC, 512], MMDT, tag="xt")
                xv = xT[:, 512 * r : 512 * (r + 1)].rearrange("(c p) t -> p c t", p=128)
                for c in range(NC):
                    (eng or nc.sync).dma_start(xt[:, c, :], xv[:, c, :])
                xts[r] = xt

            # ---- projection / output chains as (cost, emit) generators ----

            def qk_chain(r, w_sb, dst, o):
                pm = ps_mm.tile([128, 512], F32, tag="mm", bufs=2)
                for c in range(NC):
                    yield 215, lambda c=c: nc.tensor.matmul(
                        pm[:],
                        w_sb[:, c, 128 * o : 128 * (o + 1)],
                        xts[r][:, c, :],
                        start=(c == 0),
                        stop=(c == NC - 1),
                    )
                yield 0, lambda: nc.vector.tensor_copy(
                    dst[:, o, 512 * r : 512 * (r + 1)], pm[:]
                )

            def v_chain(r, tt):
                j = 4 * r + tt
                pv = ps_mm.tile([128, 512], F32, tag="mm", bufs=2)
                for c in range(NC):
                    yield 110, lambda c=c: nc.tensor.matmul(
                        pv[:, 0:GW],
                        xts[r][:, c, 128 * tt : 128 * (tt + 1)],
                        wv_sb[:, c, :],
                        start=(c == 0),
                        stop=(c == NC - 1),
                    )
                yield 0, lambda: nc.vector.tensor_copy(
                    Vt[:, j, :].rearrange("p (h u) -> p h u", u=65)[:, :, 0:64],
                    pv[:, 0:GW].rearrange("p (h d) -> p h d", d=HD),
                )

            def wo_chain(r, qq, o, alt_pool=False, tail_idx=None):
                qt = 4 * r + qq
                if alt_pool:
                    # the scores banks are dead in the tail; reuse them so
                    # four chains pipeline instead of two
                    pot = ps_sc.tile([128, 1024], F32, tag="s2", bufs=2, name="po")
                    po = pot[:, 0:512]
                else:
                    pot = ps_mm.tile([128, 512], F32, tag="mm", bufs=2, name="po")
                    po = pot[:, :]
                for d in range(2):
                    yield 215, lambda d=d: nc.tensor.matmul(
                        po,
                        ctxT[:, d, 128 * qt : 128 * (qt + 1)],
                        wo_sb[:, d, 512 * o : 512 * (o + 1)],
                        start=(d == 0), stop=(d == 1),
                    )

                def fin():
                    ot = opool.tile([128, 512], F32, tag="ot")
                    # in the tail the scalar engine is done with exps and the
                    # DMA dispatch queues are idle: spread staging copies and
                    # output descriptors across engines so chains pipeline
                    if tail_idx is not None and tail_idx % 2 == 0:
                        nc.scalar.copy(ot[:], po)
                    else:
                        nc.vector.tensor_copy(ot[:], po)
                    eng = (
                        nc.sync if tail_idx is None
                        else (nc.scalar, nc.gpsimd, nc.sync)[tail_idx % 3]
                    )
                    eng.dma_start(
                        out[128 * qt : 128 * (qt + 1), 512 * o : 512 * (o + 1)],
                        ot[:],
                    )
                yield 0, fin

            def q_chain_gens(r):
                return [qk_chain(r, wq_sb, QT, o) for o in range(2)]

            def kv_chain_gens(r):
                ch = [qk_chain(r, wk_sb, KT, o) for o in range(2)]
                ch += [v_chain(r, tt) for tt in range(4)]
                return ch

            def c_chain_gens(r):
                return [wo_chain(r, qq, o) for qq in range(4) for o in range(2)]

            # three priorities; generators always run front-to-completion so
            # psum "mm" buffer rotation matches emission order.
            kv_q = deque()      # this range's K/V chains (most urgent)
            urgent_q = deque()  # next range's Q chains (drained by boundary)
            defer_q = deque()   # Wo chains for finished ranges (lazy)

            defer_floor = [0]
            v_q = deque()
            vdone = [0]

            def pop_filler(budget):
                while budget > 0:
                    if v_q:
                        try:
                            cost, emit = next(v_q[0])
                        except StopIteration:
                            v_q.popleft()
                            vdone[0] += 1
                            continue
                        emit()
                        budget -= cost
                        continue
                    q = kv_q or urgent_q
                    if not q:
                        # wo chains read ctxT: make sure every pending
                        # normalization step is emitted first
                        while norm_q:
                            norm_q.popleft()()
                        if len(defer_q) <= defer_floor[0]:
                            return
                        q = defer_q
                    if not q:
                        return
                    try:
                        cost, emit = next(q[0])
                    except StopIteration:
                        q.popleft()
                        continue
                    emit()
                    budget -= cost

            def drain(q):
                while q:
                    try:
                        cost, emit = next(q[0])
                    except StopIteration:
                        q.popleft()
                        continue
                    emit()

            # ---- attention blocks ----

            def scores(r, p, j):
                v = j - 4 * r
                off = 128 * v if v > 0 else 0
                s2 = ps_sc.tile([128, 1024], F32, tag="s2", bufs=2)
                qs = slice(512 * r + off, 512 * (r + 1))
                nc.tensor.matmul(
                    s2[:, off:512],
                    KT[0:64, p, 128 * j : 128 * (j + 1)],
                    QT[0:64, p, qs],
                    start=True, stop=True,
                )
                nc.tensor.matmul(
                    s2[:, 512 + off : 1024],
                    KT[64:128, p, 128 * j : 128 * (j + 1)],
                    QT[64:128, p, qs],
                    start=True, stop=True,
                )
                pt2 = ppool.tile([128, 1024], MMDT, tag="pt")
                if v >= 0:
                    s2v = s2[:, :].rearrange("p (s q) -> p s q", s=2)
                    pt2v = pt2[:, :].rearrange("p (s q) -> p s q", s=2)
                    if r == NT - 1:
                        # exp-starved last range: mask BEFORE exp with a DVE
                        # add (the DVE is idle here, and the add hides behind
                        # the previous block's exp) so the post-exp gpsimd
                        # select drops off the exp->PV pacing chain
                        nc.vector.tensor_add(
                            s2v[:, :, off : off + 128],
                            s2v[:, :, off : off + 128],
                            tri2,
                        )
                        nc.scalar.activation(
                            pt2v[:, :, off:512], s2v[:, :, off:512],
                            EXPF, scale=SCALE,
                        )
                    else:
                        nc.scalar.activation(
                            pt2v[:, :, off:512], s2v[:, :, off:512],
                            EXPF, scale=SCALE,
                        )
                        # zero the 128x128 upper triangle on the diagonal
                        nc.gpsimd.affine_select(
                            out=pt2v[:, :, off : off + 128],
                            in_=pt2v[:, :, off : off + 128],
                            compare_op=mybir.AluOpType.is_ge,
                            fill=0.0,
                            base=0,
                            pattern=[[0, 2], [1, 128]],
                            channel_multiplier=-1,
                        )
                else:
                    nc.scalar.activation(pt2[:], s2[:], EXPF, scale=SCALE)
                return pt2, off

            def pv(r, p, j, pt2, off, ca, cb, nj):
                hA, hB = 2 * p, 2 * p + 1
                nc.tensor.matmul(
                    ca[:, off:512], Vt[:, j, 65 * hA : 65 * hA + 65],
                    pt2[:, off:512],
                    start=(j == 0), stop=(j == nj - 1),
                )
                nc.tensor.matmul(
                    cb[:, off:512], Vt[:, j, 65 * hB : 65 * hB + 65],
                    pt2[:, 512 + off : 1024],
                    start=(j == 0), stop=(j == nj - 1),
                )

            norm_q = deque()

            def epilogue(r, p, ca, cb):
                # stage accumulators to SBUF immediately (frees the PSUM
                # banks for the next pair) and kick off the denominator
                # bounce; the rest of the normalization chain is deferred
                # one step per block so its cross-engine latency
                # (DMA -> recip -> gpsimd broadcast -> mul) never stalls the
                # in-order DVE/gpsimd queues that the attention pipe needs.
                stA = spool.tile([65, 512], F32, tag="st")
                stB = spool.tile([65, 512], F32, tag="st")
                nc.vector.tensor_copy(stA[:], ca[:])
                nc.vector.tensor_copy(stB[:], cb[:])
                # denominator rows hop from partition 64 to partition 0 with
                # a single DVE copy (32-aligned per-operand partition bases
                # are legal) -- no DMA bounce, so the chain has no dma-sem
                # latency; then fast reciprocal + gpsimd broadcast + multiply
                srA = spool.tile([1, 512], F32, tag="sw")
                srB = spool.tile([1, 512], F32, tag="sw")
                ra = spool.tile([1, 512], F32, tag="rc")
                rb = spool.tile([1, 512], F32, tag="rc")
                bca = spool.tile([64, 512], F32, tag="bc")
                bcb = spool.tile([64, 512], F32, tag="bc")
                qs = slice(512 * r, 512 * (r + 1))
                norm_q.extend([
                    lambda: (
                        nc.vector.tensor_copy(srA[:], stA[64:65, :]),
                        nc.vector.tensor_copy(srB[:], stB[64:65, :]),
                    ),
                    lambda: (
                        nc.vector.reciprocal_approx_fast(ra[:], srA[:]),
                        nc.vector.reciprocal_approx_fast(rb[:], srB[:]),
                    ),
                    lambda: nc.gpsimd.partition_broadcast(bca[:], ra[:]),
                    lambda: nc.vector.tensor_mul(
                        ctxT[0:64, p, qs], stA[0:64, :], bca[:]
                    ),
                    lambda: nc.gpsimd.partition_broadcast(bcb[:], rb[:]),
                    lambda: nc.vector.tensor_mul(
                        ctxT[64:128, p, qs], stB[0:64, :], bcb[:]
                    ),
                ])

            # ---- prologue ----
            # weight/x loads are split into multiple descriptors and spread
            # over four engines' dma queues: each DMA_DIRECT2D costs ~600ns
            # of issuing-engine time, so a single queue serializes the
            # startup. All engines are idle here anyway.
            wqv = wq.rearrange("p (c o) -> p c o", o=GW)
            wkv = wk.rearrange("p (c o) -> p c o", o=GW)
            wvv = wv.rearrange("p (c o) -> p c o", o=GW)
            wov = wo.rearrange("p (c o) -> p c o", o=D)
            # sync: wq chunks interleaved with the x chunks in the order the
            # first projection chain consumes them, then x1 (needed last)
            xt0 = xpool.tile([128, NC, 512], MMDT, tag="xt")
            xv0 = xT[:, 0:512].rearrange("(c p) t -> p c t", p=128)
            # gpsimd dispatches the first two x chunks in parallel with
            # sync's wq0, BEFORE the gpsimd library warm-load occupies it:
            # the first projection matmul is gated on chunk 0's last byte
            nc.gpsimd.dma_start(xt0[:, 0, :], xv0[:, 0, :])
            nc.gpsimd.dma_start(xt0[:, 1, :], xv0[:, 1, :])
            # warm up the gpsimd custom-op library (partition_broadcast
            # lives in a dynamically-loaded lib; the ~7us load runs here,
            # during the initial DMA wait, not at the first epilogue)
            nc.gpsimd.partition_broadcast(warmb[:], warm[:])
            nc.gpsimd.memset(tri[:], 0.0)
            nc.gpsimd.affine_select(
                out=tri[:],
                in_=tri[:],
                compare_op=mybir.AluOpType.is_ge,
                fill=-1.0e30,
                base=0,
                pattern=[[1, 128]],
                channel_multiplier=-1,
            )
            nc.sync.dma_start(wq_sb[:, 0:1, :], wqv[:, 0:1, :])
            nc.sync.dma_start(wq_sb[:, 1:4, :], wqv[:, 1:4, :])
            for c in range(2, 4):
                nc.sync.dma_start(xt0[:, c, :], xv0[:, c, :])
            nc.sync.dma_start(wq_sb[:, 4:NC, :], wqv[:, 4:NC, :])
            for c in range(4, NC):
                nc.sync.dma_start(xt0[:, c, :], xv0[:, c, :])
            xts[0] = xt0
            # scalar: wk (gates the k chain), then wv (gates v chains), then
            # x1 -- sequenced behind the weights so its 1MB doesn't steal
            # DMA bandwidth from x0 during the startup-critical window
            for c in range(0, NC, 2):
                nc.scalar.dma_start(wk_sb[:, c : c + 2, :], wkv[:, c : c + 2, :])
            for c in range(0, NC, 2):
                nc.scalar.dma_start(wv_sb[:, c : c + 2, :], wvv[:, c : c + 2, :])
            load_xt(1)
            nc.gpsimd.dma_start(wo_sb[:, 0:1, :], wov[:, 0:1, :])
            nc.gpsimd.dma_start(wo_sb[:, 1:2, :], wov[:, 1:2, :])

            # emit only pair-0's q/k chains eagerly; the V chains drain
            # just-in-time before each PV(0,0,j), and the o=1 chains before
            # pair 1 -- so the first scores matmul and the exp pipeline
            # start ~5us earlier.
            for g in (qk_chain(0, wq_sb, QT, 0), qk_chain(0, wk_sb, KT, 0)):
                for _, emit in g:
                    emit()
            v_q.extend(v_chain(0, tt) for tt in range(4))
            kv_q.append(qk_chain(0, wq_sb, QT, 1))
            kv_q.append(qk_chain(0, wk_sb, KT, 1))

            def need_v0(j):
                # fully emit v_chain(0, 0..j) before PV(0,0,j)
                while vdone[0] <= j and v_q:
                    for _, emit in v_q[0]:
                        emit()
                    v_q.popleft()
                    vdone[0] += 1

            # ---- main loop: software-pipelined block stream ----
            kv_held = []
            for r in range(NT):
                nj = 4 * r + 4
                kv_q.extend(kv_held)
                kv_held = []
                if r + 1 < NT:
                    urgent_q.extend(q_chain_gens(r + 1))
                    kv_held = kv_chain_gens(r + 1)
                else:
                    # keep several wo chains in reserve: they feed the
                    # exp-starved last range and cover the PE through the
                    # final pair's normalization latency
                    defer_floor[0] = 6
                if r + 2 < NT:
                    load_xt(r + 2)
                if r > 0:
                    defer_q.extend(c_chain_gens(r - 1))

                blocks = [(p, j) for p in (0, 1) for j in range(nj)]
                acc = {}
                pend = {}

                def emit_scores(b):
                    # K/V chunks 4r..4r+3 come from this range's deferred
                    # chains (for r=0, pair 1's Q/K are deferred); they must
                    # be fully emitted (in-order PE queue) before any scores
                    # that reads them.
                    if (r == 0 and b[0] == 1) or (r > 0 and b[1] >= 4 * r):
                        drain(kv_q)
                    pend[b] = scores(r, *b)

                emit_scores(blocks[0])
                for i, (p, j) in enumerate(blocks):
                    if i + 1 < len(blocks):
                        emit_scores(blocks[i + 1])
                    if norm_q:
                        norm_q.popleft()()
                    pop_filler(500)
                    if j == 0:
                        acc[p] = (
                            ps_acc.tile([65, 512], F32, tag="acc", bufs=2, name="ca"),
                            ps_acc.tile([65, 512], F32, tag="acc", bufs=2, name="cb"),
                        )
                    if r == 0 and p == 0:
                        need_v0(j)
                    pt2, off = pend.pop((p, j))
                    pv(r, p, j, pt2, off, *acc[p], nj)
                    if j == nj - 1:
                        epilogue(r, p, *acc[p])
                drain(kv_q)
                drain(urgent_q)
            # ---- tail: wo chains for the last range ----
            # PE cover for the last pair's normalization latency: first the
            # reserved wo chains of range NT-2 (fully ready), then the d=0
            # matmuls of the two s2-pool chains (depend only on pair-0 ctx).
            # The s2 banks are dead here, so these don't collide with the
            # norm broadcasts' mm-pool tiles.
            defer_floor[0] = 0
            print("TAIL-RESERVES:", len(defer_q))
            drain(defer_q)
            tail = [
                wo_chain(NT - 1, qq, o, alt_pool=bool((2 * qq + o) % 2),
                         tail_idx=2 * qq + o)
                for qq in range(4) for o in range(2)
            ]
            pre = [g for g in tail if tail.index(g) % 2 == 1][:2]
            for g in pre:
                cost, emit = next(g)
                emit()
            while norm_q:
                norm_q.popleft()()
            defer_q.extend(tail)
            drain(defer_q)

    nc.compile()
    return nc


def _get_nc():
    if "nc" not in _CACHE:
        _CACHE["nc"] = _build()
    return _CACHE["nc"]


def kernel(x, Wq, Wk, Wv, Wo, bo):
    global LAST_RESULTS
    x = np.asarray(x, dtype=np.float32)
    Wq = np.asarray(Wq, dtype=np.float32)
    Wk = np.asarray(Wk, dtype=np.float32)
    Wv = np.asarray(Wv, dtype=np.float32)
    Wo = np.asarray(Wo, dtype=np.float32)
    bo = np.asarray(bo, dtype=np.float32)

    nc = _get_nc()
    xTs = [np.ascontiguousarray(x[b].T).astype(NPDT) for b in range(B)]

    def warr(w, cs):
        # [D, GW] slice -> [128, NC*GW]: partition p holds chunk-major rows
        s = w[:, cs].reshape(D // 128, 128, GW).transpose(1, 0, 2)
        return np.ascontiguousarray(s.reshape(128, -1)).astype(NPDT)

    def woarr(cs):
        # [GW, D] slice -> [128, 2*D]
        s = Wo[cs, :].reshape(GW // 128, 128, D).transpose(1, 0, 2)
        return np.ascontiguousarray(s.reshape(128, -1)).astype(NPDT)

    in_maps = []
    for c in range(N_CORES):
        b, g = divmod(c, N_CORES // B)
        cs = slice(GW * g, GW * (g + 1))
        in_maps.append(
            {
                "xT": xTs[b],
                "wq": warr(Wq, cs),
                "wk": warr(Wk, cs),
                "wv": warr(Wv, cs),
                "wo": woarr(cs),
            }
        )

    _maybe_install_trace_hook()
    res = bass_utils.run_bass_kernel_spmd(nc, in_maps, core_ids=list(range(N_CORES)))
    LAST_RESULTS = res

    out = np.zeros((B, S, D), dtype=np.float32)
    for c in range(N_CORES):
        out[c // (N_CORES // B)] += res.results[c]["out"]
    out += bo[None, None, :]
    return out
